# revision 15
# baseline (speedup 1.0000x reference)
"""Trainium2 Bass kernel for nn_DecodeAttention (B=32, S=4095, E=1024, H=16).

Contract: kernel(**full_inputs) -> (out, new_k, new_v) as full-shape numpy
arrays. Internally shards batch across 8 NeuronCores (4 batches/core), runs
one SPMD Bass/Tile program via run_bass_kernel_spmd, gathers results.

Self-contained: hardcodes shapes; only imports the concourse toolchain from
its installed location.
"""

import os
import sys

if "/opt/trn_rl_repo" not in sys.path:
    sys.path.insert(0, "/opt/trn_rl_repo")

import numpy as np

B, S, E, H = 32, 4095, 1024, 16
HD = E // H            # 64
SEQ = S + 1            # 4096
NCORES = 8
BPC = B // NCORES      # 4 batches per core
CHUNK = 128            # rows per score sub-chunk (SBUF partitions)
NCHUNK = SEQ // CHUNK  # 32 sub-chunks per batch
CPB = 4                # sub-chunks per DMA block
SBLK = CHUNK * CPB     # 512 rows per DMA block
NBLK = NCHUNK // CPB   # 8 blocks per batch
NGRP = E // CHUNK      # 8 contraction groups of 128

_compiled = {}
LAST_EXEC_NS = None
LAST_RESULTS = None


def _build(lasts, mask_all_ones):
    """Build + compile the SPMD program for one core's 4 batches.

    lasts: tuple of BPC ints — scatter position per batch (baked into APs).
    mask_all_ones: if False, a maskcols input is declared and exp(scores) is
    multiplied by the per-position mask (equivalent to the -1e9 fill).
    """
    import concourse.tile as tile
    from concourse import bacc, mybir
    from concourse.masks import make_identity

    f32 = mybir.dt.float32
    AF = mybir.ActivationFunctionType
    AX = mybir.AxisListType

    nc = bacc.Bacc(
        "TRN2",
        target_bir_lowering=False,
        debug=False,
        enable_asserts=False,
        num_devices=NCORES,
    )

    kc = nc.dram_tensor("kcache", [BPC, S, E], f32, kind="ExternalInput")
    vc = nc.dram_tensor("vcache", [BPC, S, E], f32, kind="ExternalInput")
    hst = nc.dram_tensor("hst", [E, BPC], f32, kind="ExternalInput")
    wattn = nc.dram_tensor("wattn", [E, 3 * E], f32, kind="ExternalInput")
    battn = nc.dram_tensor("battn", [BPC, 3 * E], f32, kind="ExternalInput")
    wproj = nc.dram_tensor("wproj", [E, E], f32, kind="ExternalInput")
    bproj = nc.dram_tensor("bproj", [BPC, E], f32, kind="ExternalInput")
    biasc = nc.dram_tensor("biascols", [CHUNK, NCHUNK], f32, kind="ExternalInput")
    maskc = None
    if not mask_all_ones:
        maskc = nc.dram_tensor(
            "maskcols", [BPC, CHUNK, NCHUNK], f32, kind="ExternalInput"
        )

    newk = nc.dram_tensor("newk", [BPC, SEQ, E], f32, kind="ExternalOutput")
    newv = nc.dram_tensor("newv", [BPC, SEQ, E], f32, kind="ExternalOutput")
    yout = nc.dram_tensor("yout", [BPC, E], f32, kind="ExternalOutput")

    with tile.TileContext(nc) as tc:
        with (
            tc.tile_pool(name="consts", bufs=1) as consts,
            tc.tile_pool(name="wpool", bufs=1) as wpool,
            tc.tile_pool(name="kv", bufs=2) as kvpool,
            tc.tile_pool(name="tmp", bufs=3) as tmppool,
            tc.tile_pool(name="small", bufs=4) as small,
            tc.tile_pool(name="qp", bufs=2) as qpool,
        ):
            # ---- constants ----
            ones_col = consts.tile([CHUNK, 1], f32)
            nc.gpsimd.memset(ones_col[:], 1.0)
            ident = consts.tile([16, 16], f32)
            make_identity(nc, ident[:])
            biast = consts.tile([CHUNK, NCHUNK], f32)
            nc.sync.dma_start(biast[:], biasc[:])
            battn_t = consts.tile([BPC, 3 * E], f32)
            nc.sync.dma_start(battn_t[:], battn[:])
            bproj_t = consts.tile([BPC, E], f32)
            nc.sync.dma_start(bproj_t[:], bproj[:])
            hst_t = consts.tile([CHUNK, NGRP, BPC], f32)
            nc.sync.dma_start(
                hst_t[:], hst[:, :].rearrange("(g p) b -> p g b", p=CHUNK)
            )
            wproj_t = consts.tile([CHUNK, NGRP, E], f32)
            nc.sync.dma_start(
                wproj_t[:], wproj[:, :].rearrange("(g p) f -> p g f", p=CHUNK)
            )
            ctxT = consts.tile([CHUNK, NGRP * BPC], f32)
            qkv_sb = consts.tile([BPC, 3 * E], f32)

            # ---- fused qkv projection: qkv = hs @ Wattn + b ----
            with tc.tile_pool(name="psqkv", bufs=1, space="PSUM") as psqkv:
                qkv_ps = psqkv.tile([BPC, 3 * E], f32, tag="qkv")
                for g in range(NGRP):
                    wt = wpool.tile([CHUNK, 3 * E], f32)
                    nc.sync.dma_start(
                        wt[:], wattn[g * CHUNK : (g + 1) * CHUNK, :]
                    )
                    for n in range(6):
                        nc.tensor.matmul(
                            qkv_ps[:, n * 512 : (n + 1) * 512],
                            hst_t[:, g, :],
                            wt[:, n * 512 : (n + 1) * 512],
                            start=(g == 0),
                            stop=(g == NGRP - 1),
                        )
                nc.vector.tensor_add(qkv_sb[:], qkv_ps[:], battn_t[:])

            # ---- per-batch attention + cache update ----
            psctx_cm = tc.tile_pool(name="psctx", bufs=1, space="PSUM")
            psden_cm = tc.tile_pool(name="psden", bufs=1, space="PSUM")
            pstr_cm = tc.tile_pool(name="pstr", bufs=2, space="PSUM")
            psctx = psctx_cm.__enter__()
            psden = psden_cm.__enter__()
            pstr = pstr_cm.__enter__()
            for b in range(BPC):
                last = int(lasts[b])
                lblk, lrem = last // SBLK, last % SBLK
                # layout within a block: row r0 + p*CPB + r -> kt[p, r, :]
                lpart, lchunk = lrem // CPB, lrem % CPB

                # replicate q_b across 128 partitions
                qstage = qpool.tile([1, E], f32, tag="qstage")
                nc.sync.dma_start(qstage[:], qkv_sb[b : b + 1, 0:E])
                q_rep = qpool.tile([CHUNK, E], f32, tag="qrep")
                nc.gpsimd.partition_broadcast(q_rep[:], qstage[:])

                if maskc is not None:
                    mk = qpool.tile([CHUNK, NCHUNK], f32, tag="mask")
                    nc.sync.dma_start(mk[:], maskc[b])

                ctx_ps = psctx.tile([16, E], f32)
                den_ps = psden.tile([16, 1], f32)

                for blk in range(NBLK):
                    r0 = blk * SBLK
                    kt = kvpool.tile([CHUNK, CPB, E], f32, tag="kt")
                    vt = kvpool.tile([CHUNK, CPB, E], f32, tag="vt")
                    if blk < NBLK - 1:
                        nc.sync.dma_start(
                            kt[:],
                            kc[b, r0 : r0 + SBLK, :].rearrange(
                                "(p r) f -> p r f", p=CHUNK
                            ),
                        )
                        nc.sync.dma_start(
                            vt[:],
                            vc[b, r0 : r0 + SBLK, :].rearrange(
                                "(p r) f -> p r f", p=CHUNK
                            ),
                        )
                    else:
                        # last block: 511 cache rows + row SEQ-1 filled below.
                        # partitions 0..126 hold 4 consecutive rows each;
                        # partition 127 holds rows 508..510 (+ appended row).
                        nfull = SBLK - CPB  # 508
                        nc.sync.dma_start(
                            kt[0 : CHUNK - 1, :, :],
                            kc[b, r0 : r0 + nfull, :].rearrange(
                                "(p r) f -> p r f", p=CHUNK - 1
                            ),
                        )
                        nc.sync.dma_start(
                            kt[CHUNK - 1 : CHUNK, 0 : CPB - 1, :],
                            kc[b, r0 + nfull : S, :].rearrange(
                                "(p r) f -> p r f", p=1
                            ),
                        )
                        nc.sync.dma_start(
                            vt[0 : CHUNK - 1, :, :],
                            vc[b, r0 : r0 + nfull, :].rearrange(
                                "(p r) f -> p r f", p=CHUNK - 1
                            ),
                        )
                        nc.sync.dma_start(
                            vt[CHUNK - 1 : CHUNK, 0 : CPB - 1, :],
                            vc[b, r0 + nfull : S, :].rearrange(
                                "(p r) f -> p r f", p=1
                            ),
                        )
                        if last != SEQ - 1:
                            # appended row stays zero unless scattered into
                            nc.gpsimd.memset(kt[CHUNK - 1 : CHUNK, CPB - 1, :], 0.0)
                            nc.gpsimd.memset(vt[CHUNK - 1 : CHUNK, CPB - 1, :], 0.0)
                    if blk == lblk:
                        # scatter new k/v row at position `last`
                        nc.sync.dma_start(
                            kt[lpart : lpart + 1, lchunk, :],
                            qkv_sb[b : b + 1, E : 2 * E],
                        )
                        nc.sync.dma_start(
                            vt[lpart : lpart + 1, lchunk, :],
                            qkv_sb[b : b + 1, 2 * E : 3 * E],
                        )

                    for c in range(CPB):
                        cg = blk * CPB + c
                        tmp = tmppool.tile([CHUNK, E], f32)
                        nc.vector.tensor_mul(tmp[:], kt[:, c, :], q_rep[:])
                        sc = small.tile([CHUNK, H], f32, tag="sc")
                        nc.vector.reduce_sum(
                            sc[:],
                            tmp[:].rearrange("p (h d) -> p h d", h=H),
                            axis=AX.X,
                        )
                        ex = small.tile([CHUNK, H], f32, tag="ex")
                        nc.scalar.activation(
                            ex[:],
                            sc[:],
                            AF.Exp,
                            bias=biast[:, cg : cg + 1],
                            scale=1.0 / 8.0,
                        )
                        if maskc is not None:
                            nc.vector.tensor_scalar_mul(
                                ex[:], ex[:], mk[:, cg : cg + 1]
                            )
                        st = cg == 0
                        sp = cg == NCHUNK - 1
                        nc.tensor.matmul(
                            ctx_ps[:, 0:512], ex[:], vt[:, c, 0:512],
                            start=st, stop=sp,
                        )
                        nc.tensor.matmul(
                            ctx_ps[:, 512:E], ex[:], vt[:, c, 512:E],
                            start=st, stop=sp,
                        )
                        nc.tensor.matmul(
                            den_ps[:], ex[:], ones_col[:], start=st, stop=sp
                        )

                    nc.sync.dma_start(
                        newk[b, r0 : r0 + SBLK, :].rearrange(
                            "(p r) f -> p r f", p=CHUNK
                        ),
                        kt[:],
                    )
                    nc.sync.dma_start(
                        newv[b, r0 : r0 + SBLK, :].rearrange(
                            "(p r) f -> p r f", p=CHUNK
                        ),
                        vt[:],
                    )

                # normalize: ctx[h, :] * (1 / denom[h]) via per-partition scale
                recip = small.tile([16, 1], f32, tag="recip", bufs=2)
                nc.vector.reciprocal(recip[:], den_ps[:])
                ctxn = small.tile([16, E], f32, tag="ctxn", bufs=2)
                nc.scalar.activation(
                    ctxn[:], ctx_ps[:], AF.Copy, bias=0.0, scale=recip[:]
                )
                # transpose [16, E] -> ctxT columns, keeping only each head's
                # own 64-dim block (diagonal extraction)
                for g in range(NGRP):
                    trp = pstr.tile([CHUNK, 16], f32)
                    nc.tensor.transpose(
                        trp[:], ctxn[:, g * CHUNK : (g + 1) * CHUNK], ident[:]
                    )
                    col = g * BPC + b
                    nc.vector.tensor_copy(
                        ctxT[0:HD, col : col + 1], trp[0:HD, 2 * g : 2 * g + 1]
                    )
                    nc.vector.tensor_copy(
                        ctxT[HD:CHUNK, col : col + 1],
                        trp[HD:CHUNK, 2 * g + 1 : 2 * g + 2],
                    )

            # ---- output projection for all 4 batches ----
            with tc.tile_pool(name="psy", bufs=1, space="PSUM") as psy:
                y_ps = psy.tile([BPC, E], f32, tag="y")
                for g in range(NGRP):
                    nc.tensor.matmul(
                        y_ps[:, 0:512],
                        ctxT[:, g * BPC : (g + 1) * BPC],
                        wproj_t[:, g, 0:512],
                        start=(g == 0),
                        stop=(g == NGRP - 1),
                    )
                    nc.tensor.matmul(
                        y_ps[:, 512:E],
                        ctxT[:, g * BPC : (g + 1) * BPC],
                        wproj_t[:, g, 512:E],
                        start=(g == 0),
                        stop=(g == NGRP - 1),
                    )
                y_sb = consts.tile([BPC, E], f32, tag="ysb")
                nc.vector.tensor_add(y_sb[:], y_ps[:], bproj_t[:])
                nc.sync.dma_start(yout[:], y_sb[:])
            pstr_cm.__exit__(None, None, None)
            psden_cm.__exit__(None, None, None)
            psctx_cm.__exit__(None, None, None)

    nc.compile()
    return nc


def get_program(lasts, mask_all_ones):
    key = (tuple(int(x) for x in lasts), bool(mask_all_ones))
    if key not in _compiled:
        _compiled[key] = _build(key[0], key[1])
    return _compiled[key]


def kernel(
    hidden_states,
    key_cache,
    value_cache,
    attention_mask,
    c_attn_w,
    c_attn_b,
    c_proj_w,
    c_proj_b,
    attn_bias,
):
    global LAST_EXEC_NS, LAST_RESULTS

    hs = np.ascontiguousarray(np.asarray(hidden_states, dtype=np.float32))
    kcache = np.ascontiguousarray(np.asarray(key_cache, dtype=np.float32))
    vcache = np.ascontiguousarray(np.asarray(value_cache, dtype=np.float32))
    mask = np.asarray(attention_mask)
    w_attn = np.ascontiguousarray(np.asarray(c_attn_w, dtype=np.float32))
    b_attn = np.asarray(c_attn_b, dtype=np.float32)
    w_proj = np.ascontiguousarray(np.asarray(c_proj_w, dtype=np.float32))
    b_proj = np.asarray(c_proj_b, dtype=np.float32)
    bias = np.asarray(attn_bias, dtype=np.float32)

    # last valid position per batch (index of last 1 in the mask)
    rev = mask[:, ::-1]
    last = mask.shape[1] - np.argmax(rev, axis=1) - 1
    mask_all_ones = bool((mask == 1).all())
    lasts_by_core = last.reshape(NCORES, BPC)
    if not (lasts_by_core == lasts_by_core[0]).all():
        raise NotImplementedError(
            "scatter positions must match across batch shards (SPMD program)"
        )

    nc = get_program(lasts_by_core[0], mask_all_ones)

    hstT = np.ascontiguousarray(hs[:, 0, :].T)  # [E, B]
    # biast[p, blk*CPB + r] = attn_bias[0, blk*SBLK + p*CPB + r]
    biascols = np.ascontiguousarray(
        bias[0, :SEQ].reshape(NBLK, CHUNK, CPB).transpose(1, 0, 2).reshape(CHUNK, NCHUNK)
    )
    battn = np.ascontiguousarray(np.tile(b_attn[None, :], (BPC, 1)))
    bprojt = np.ascontiguousarray(np.tile(b_proj[None, :], (BPC, 1)))
    if not mask_all_ones:
        maskcols = np.ascontiguousarray(
            mask[:, :SEQ]
            .reshape(B, NBLK, CHUNK, CPB)
            .transpose(0, 2, 1, 3)
            .reshape(B, CHUNK, NCHUNK)
            .astype(np.float32)
        )

    in_maps = []
    for i in range(NCORES):
        sl = slice(i * BPC, (i + 1) * BPC)
        m = {
            "kcache": kcache[sl],
            "vcache": vcache[sl],
            "hst": np.ascontiguousarray(hstT[:, sl]),
            "wattn": w_attn,
            "battn": battn,
            "wproj": w_proj,
            "bproj": bprojt,
            "biascols": biascols,
        }
        if not mask_all_ones:
            m["maskcols"] = maskcols[sl]
        in_maps.append(m)

    from concourse.bass_utils import run_bass_kernel_spmd

    trace = bool(int(os.environ.get("KERNEL_TRACE", "0")))
    res = run_bass_kernel_spmd(
        nc, in_maps, list(range(NCORES)), trace=trace
    )
    LAST_EXEC_NS = res.exec_time_ns
    LAST_RESULTS = res

    out = np.empty((B, 1, E), np.float32)
    new_k = np.empty((B, SEQ, E), np.float32)
    new_v = np.empty((B, SEQ, E), np.float32)
    for i, r in enumerate(res.results):
        sl = slice(i * BPC, (i + 1) * BPC)
        out[sl, 0, :] = r["yout"]
        new_k[sl] = r["newk"]
        new_v[sl] = r["newv"]
    return out, new_k, new_v


# revision 17
# speedup vs baseline: 1.5261x; 1.5261x over previous
"""Trainium2 Bass kernel for nn_DecodeAttention (B=32, S=4095, E=1024, H=16).

Contract: kernel(**full_inputs) -> (out, new_k, new_v) as full-shape numpy
arrays. Internally shards batch across 8 NeuronCores (4 batches/core), runs
one SPMD Bass/Tile program via run_bass_kernel_spmd, gathers results.

Self-contained: hardcodes shapes; only imports the concourse toolchain from
its installed location.
"""

import os
import sys

if "/opt/trn_rl_repo" not in sys.path:
    sys.path.insert(0, "/opt/trn_rl_repo")

import numpy as np

B, S, E, H = 32, 4095, 1024, 16
HD = E // H            # 64
SEQ = S + 1            # 4096
NCORES = 8
BPC = B // NCORES      # 4 batches per core
CHUNK = 128            # rows per score sub-chunk (SBUF partitions)
NCHUNK = SEQ // CHUNK  # 32 sub-chunks per batch
CPB = 4                # sub-chunks per DMA block
SBLK = CHUNK * CPB     # 512 rows per DMA block
NBLK = NCHUNK // CPB   # 8 blocks per batch
NGRP = E // CHUNK      # 8 contraction groups of 128

_compiled = {}
LAST_EXEC_NS = None
LAST_RESULTS = None


def _build(lasts, mask_all_ones):
    """Build + compile the SPMD program for one core's 4 batches.

    lasts: tuple of BPC ints — scatter position per batch (baked into APs).
    mask_all_ones: if False, a maskcols input is declared and exp(scores) is
    multiplied by the per-position mask (equivalent to the -1e9 fill).
    """
    import concourse.tile as tile
    from concourse import bacc, mybir
    from concourse.masks import make_identity

    f32 = mybir.dt.float32
    AF = mybir.ActivationFunctionType
    AX = mybir.AxisListType

    nc = bacc.Bacc(
        "TRN2",
        target_bir_lowering=False,
        debug=False,
        enable_asserts=False,
        num_devices=NCORES,
    )

    kc = nc.dram_tensor("kcache", [BPC, SEQ, E], f32, kind="ExternalInput")
    vc = nc.dram_tensor("vcache", [BPC, SEQ, E], f32, kind="ExternalInput")
    hst = nc.dram_tensor("hst", [E, BPC], f32, kind="ExternalInput")
    wattn = nc.dram_tensor("wattn", [E, 3 * E], f32, kind="ExternalInput")
    battn = nc.dram_tensor("battn", [BPC, 3 * E], f32, kind="ExternalInput")
    wproj = nc.dram_tensor("wproj", [E, E], f32, kind="ExternalInput")
    bproj = nc.dram_tensor("bproj", [BPC, E], f32, kind="ExternalInput")
    biasc = nc.dram_tensor("biascols", [CHUNK, NCHUNK], f32, kind="ExternalInput")
    maskc = None
    if not mask_all_ones:
        maskc = nc.dram_tensor(
            "maskcols", [BPC, CHUNK, NCHUNK], f32, kind="ExternalInput"
        )

    newk = nc.dram_tensor("newk", [BPC, SEQ, E], f32, kind="ExternalOutput")
    newv = nc.dram_tensor("newv", [BPC, SEQ, E], f32, kind="ExternalOutput")
    yout = nc.dram_tensor("yout", [BPC, E], f32, kind="ExternalOutput")

    with tile.TileContext(nc) as tc:
        with (
            tc.tile_pool(name="consts", bufs=1) as consts,
            tc.tile_pool(name="wpool", bufs=1) as wpool,
            tc.tile_pool(name="kv", bufs=2) as kvpool,
            tc.tile_pool(name="tmp", bufs=3) as tmppool,
            tc.tile_pool(name="small", bufs=4) as small,
            tc.tile_pool(name="qp", bufs=2) as qpool,
        ):
            # ---- constants ----
            ones_col = consts.tile([CHUNK, 1], f32)
            nc.gpsimd.memset(ones_col[:], 1.0)
            ident = consts.tile([16, 16], f32)
            make_identity(nc, ident[:])
            biast = consts.tile([CHUNK, NCHUNK], f32)
            nc.sync.dma_start(biast[:], biasc[:])
            battn_t = consts.tile([BPC, 3 * E], f32)
            nc.sync.dma_start(battn_t[:], battn[:])
            bproj_t = consts.tile([BPC, E], f32)
            nc.sync.dma_start(bproj_t[:], bproj[:])
            hst_t = consts.tile([CHUNK, NGRP, BPC], f32)
            nc.sync.dma_start(
                hst_t[:], hst[:, :].rearrange("(g p) b -> p g b", p=CHUNK)
            )
            wproj_t = consts.tile([CHUNK, NGRP, E], f32)
            nc.sync.dma_start(
                wproj_t[:], wproj[:, :].rearrange("(g p) f -> p g f", p=CHUNK)
            )
            ctxT = consts.tile([CHUNK, NGRP * BPC], f32)
            qkv_sb = consts.tile([BPC, 3 * E], f32)

            # ---- fused qkv projection: qkv = hs @ Wattn + b ----
            with tc.tile_pool(name="psqkv", bufs=1, space="PSUM") as psqkv:
                qkv_ps = psqkv.tile([BPC, 3 * E], f32, tag="qkv")
                for g in range(NGRP):
                    wt = wpool.tile([CHUNK, 3 * E], f32)
                    nc.sync.dma_start(
                        wt[:], wattn[g * CHUNK : (g + 1) * CHUNK, :]
                    )
                    for n in range(6):
                        nc.tensor.matmul(
                            qkv_ps[:, n * 512 : (n + 1) * 512],
                            hst_t[:, g, :],
                            wt[:, n * 512 : (n + 1) * 512],
                            start=(g == 0),
                            stop=(g == NGRP - 1),
                        )
                nc.vector.tensor_add(qkv_sb[:], qkv_ps[:], battn_t[:])

            # ---- per-batch attention + cache update ----
            psctx_cm = tc.tile_pool(name="psctx", bufs=1, space="PSUM")
            psden_cm = tc.tile_pool(name="psden", bufs=1, space="PSUM")
            pstr_cm = tc.tile_pool(name="pstr", bufs=2, space="PSUM")
            psctx = psctx_cm.__enter__()
            psden = psden_cm.__enter__()
            pstr = pstr_cm.__enter__()
            for b in range(BPC):
                last = int(lasts[b])
                lblk, lrem = last // SBLK, last % SBLK
                # layout within a block: row r0 + p*CPB + r -> kt[p, r, :]
                lpart, lchunk = lrem // CPB, lrem % CPB

                # replicate q_b across 128 partitions
                qstage = qpool.tile([1, E], f32, tag="qstage")
                nc.sync.dma_start(qstage[:], qkv_sb[b : b + 1, 0:E])
                q_rep = qpool.tile([CHUNK, E], f32, tag="qrep")
                nc.gpsimd.partition_broadcast(q_rep[:], qstage[:])

                if maskc is not None:
                    mk = qpool.tile([CHUNK, NCHUNK], f32, tag="mask")
                    nc.sync.dma_start(mk[:], maskc[b])

                ctx_ps = psctx.tile([16, E], f32)
                den_ps = psden.tile([16, 1], f32)

                for blk in range(NBLK):
                    r0 = blk * SBLK
                    kt = kvpool.tile([CHUNK, CPB, E], f32, tag="kt")
                    vt = kvpool.tile([CHUNK, CPB, E], f32, tag="vt")
                    # caches are host-padded to SEQ rows, so every block is a
                    # clean 128-partition DMA (16KB/partition descriptors).
                    nc.sync.dma_start(
                        kt[:],
                        kc[b, r0 : r0 + SBLK, :].rearrange(
                            "(p r) f -> p r f", p=CHUNK
                        ),
                    )
                    nc.sync.dma_start(
                        vt[:],
                        vc[b, r0 : r0 + SBLK, :].rearrange(
                            "(p r) f -> p r f", p=CHUNK
                        ),
                    )
                    if blk == NBLK - 1 and last != SEQ - 1:
                        # appended row must be zero unless scattered into
                        nc.gpsimd.memset(kt[CHUNK - 1 : CHUNK, CPB - 1, :], 0.0)
                        nc.gpsimd.memset(vt[CHUNK - 1 : CHUNK, CPB - 1, :], 0.0)
                    if blk == lblk:
                        # scatter new k/v row at position `last`
                        nc.sync.dma_start(
                            kt[lpart : lpart + 1, lchunk, :],
                            qkv_sb[b : b + 1, E : 2 * E],
                        )
                        nc.sync.dma_start(
                            vt[lpart : lpart + 1, lchunk, :],
                            qkv_sb[b : b + 1, 2 * E : 3 * E],
                        )

                    for c in range(CPB):
                        cg = blk * CPB + c
                        tmp = tmppool.tile([CHUNK, E], f32)
                        nc.vector.tensor_mul(tmp[:], kt[:, c, :], q_rep[:])
                        sc = small.tile([CHUNK, H], f32, tag="sc")
                        nc.vector.reduce_sum(
                            sc[:],
                            tmp[:].rearrange("p (h d) -> p h d", h=H),
                            axis=AX.X,
                        )
                        ex = small.tile([CHUNK, H], f32, tag="ex")
                        nc.scalar.activation(
                            ex[:],
                            sc[:],
                            AF.Exp,
                            bias=biast[:, cg : cg + 1],
                            scale=1.0 / 8.0,
                        )
                        if maskc is not None:
                            nc.vector.tensor_scalar_mul(
                                ex[:], ex[:], mk[:, cg : cg + 1]
                            )
                        st = cg == 0
                        sp = cg == NCHUNK - 1
                        nc.tensor.matmul(
                            ctx_ps[:, 0:512], ex[:], vt[:, c, 0:512],
                            start=st, stop=sp,
                        )
                        nc.tensor.matmul(
                            ctx_ps[:, 512:E], ex[:], vt[:, c, 512:E],
                            start=st, stop=sp,
                        )
                        nc.tensor.matmul(
                            den_ps[:], ex[:], ones_col[:], start=st, stop=sp
                        )

                    nc.scalar.dma_start(
                        newk[b, r0 : r0 + SBLK, :].rearrange(
                            "(p r) f -> p r f", p=CHUNK
                        ),
                        kt[:],
                    )
                    nc.scalar.dma_start(
                        newv[b, r0 : r0 + SBLK, :].rearrange(
                            "(p r) f -> p r f", p=CHUNK
                        ),
                        vt[:],
                    )

                # normalize: ctx[h, :] * (1 / denom[h]) via per-partition scale
                recip = small.tile([16, 1], f32, tag="recip", bufs=2)
                nc.vector.reciprocal(recip[:], den_ps[:])
                ctxn = small.tile([16, E], f32, tag="ctxn", bufs=2)
                nc.scalar.activation(
                    ctxn[:], ctx_ps[:], AF.Copy, bias=0.0, scale=recip[:]
                )
                # transpose [16, E] -> ctxT columns, keeping only each head's
                # own 64-dim block (diagonal extraction)
                for g in range(NGRP):
                    trp = pstr.tile([CHUNK, 16], f32)
                    nc.tensor.transpose(
                        trp[:], ctxn[:, g * CHUNK : (g + 1) * CHUNK], ident[:]
                    )
                    col = g * BPC + b
                    nc.vector.tensor_copy(
                        ctxT[0:HD, col : col + 1], trp[0:HD, 2 * g : 2 * g + 1]
                    )
                    nc.vector.tensor_copy(
                        ctxT[HD:CHUNK, col : col + 1],
                        trp[HD:CHUNK, 2 * g + 1 : 2 * g + 2],
                    )

            # ---- output projection for all 4 batches ----
            with tc.tile_pool(name="psy", bufs=1, space="PSUM") as psy:
                y_ps = psy.tile([BPC, E], f32, tag="y")
                for g in range(NGRP):
                    nc.tensor.matmul(
                        y_ps[:, 0:512],
                        ctxT[:, g * BPC : (g + 1) * BPC],
                        wproj_t[:, g, 0:512],
                        start=(g == 0),
                        stop=(g == NGRP - 1),
                    )
                    nc.tensor.matmul(
                        y_ps[:, 512:E],
                        ctxT[:, g * BPC : (g + 1) * BPC],
                        wproj_t[:, g, 512:E],
                        start=(g == 0),
                        stop=(g == NGRP - 1),
                    )
                y_sb = consts.tile([BPC, E], f32, tag="ysb")
                nc.vector.tensor_add(y_sb[:], y_ps[:], bproj_t[:])
                nc.sync.dma_start(yout[:], y_sb[:])
            pstr_cm.__exit__(None, None, None)
            psden_cm.__exit__(None, None, None)
            psctx_cm.__exit__(None, None, None)

    nc.compile()
    return nc


def get_program(lasts, mask_all_ones):
    key = (tuple(int(x) for x in lasts), bool(mask_all_ones))
    if key not in _compiled:
        _compiled[key] = _build(key[0], key[1])
    return _compiled[key]


def kernel(
    hidden_states,
    key_cache,
    value_cache,
    attention_mask,
    c_attn_w,
    c_attn_b,
    c_proj_w,
    c_proj_b,
    attn_bias,
):
    global LAST_EXEC_NS, LAST_RESULTS

    hs = np.ascontiguousarray(np.asarray(hidden_states, dtype=np.float32))
    # pad caches to SEQ rows so every device DMA is a uniform full-partition
    # transfer (the pad row is overwritten in SBUF by the scattered k/v row)
    kcache = np.zeros((B, SEQ, E), np.float32)
    kcache[:, :S] = np.asarray(key_cache, dtype=np.float32)
    vcache = np.zeros((B, SEQ, E), np.float32)
    vcache[:, :S] = np.asarray(value_cache, dtype=np.float32)
    mask = np.asarray(attention_mask)
    w_attn = np.ascontiguousarray(np.asarray(c_attn_w, dtype=np.float32))
    b_attn = np.asarray(c_attn_b, dtype=np.float32)
    w_proj = np.ascontiguousarray(np.asarray(c_proj_w, dtype=np.float32))
    b_proj = np.asarray(c_proj_b, dtype=np.float32)
    bias = np.asarray(attn_bias, dtype=np.float32)

    # last valid position per batch (index of last 1 in the mask)
    rev = mask[:, ::-1]
    last = mask.shape[1] - np.argmax(rev, axis=1) - 1
    mask_all_ones = bool((mask == 1).all())
    lasts_by_core = last.reshape(NCORES, BPC)
    if not (lasts_by_core == lasts_by_core[0]).all():
        raise NotImplementedError(
            "scatter positions must match across batch shards (SPMD program)"
        )

    nc = get_program(lasts_by_core[0], mask_all_ones)

    hstT = np.ascontiguousarray(hs[:, 0, :].T)  # [E, B]
    # biast[p, blk*CPB + r] = attn_bias[0, blk*SBLK + p*CPB + r]
    biascols = np.ascontiguousarray(
        bias[0, :SEQ].reshape(NBLK, CHUNK, CPB).transpose(1, 0, 2).reshape(CHUNK, NCHUNK)
    )
    battn = np.ascontiguousarray(np.tile(b_attn[None, :], (BPC, 1)))
    bprojt = np.ascontiguousarray(np.tile(b_proj[None, :], (BPC, 1)))
    if not mask_all_ones:
        maskcols = np.ascontiguousarray(
            mask[:, :SEQ]
            .reshape(B, NBLK, CHUNK, CPB)
            .transpose(0, 2, 1, 3)
            .reshape(B, CHUNK, NCHUNK)
            .astype(np.float32)
        )

    in_maps = []
    for i in range(NCORES):
        sl = slice(i * BPC, (i + 1) * BPC)
        m = {
            "kcache": kcache[sl],
            "vcache": vcache[sl],
            "hst": np.ascontiguousarray(hstT[:, sl]),
            "wattn": w_attn,
            "battn": battn,
            "wproj": w_proj,
            "bproj": bprojt,
            "biascols": biascols,
        }
        if not mask_all_ones:
            m["maskcols"] = maskcols[sl]
        in_maps.append(m)

    from concourse.bass_utils import run_bass_kernel_spmd

    trace = bool(int(os.environ.get("KERNEL_TRACE", "0")))
    res = run_bass_kernel_spmd(
        nc, in_maps, list(range(NCORES)), trace=trace
    )
    LAST_EXEC_NS = res.exec_time_ns
    LAST_RESULTS = res

    out = np.empty((B, 1, E), np.float32)
    new_k = np.empty((B, SEQ, E), np.float32)
    new_v = np.empty((B, SEQ, E), np.float32)
    for i, r in enumerate(res.results):
        sl = slice(i * BPC, (i + 1) * BPC)
        out[sl, 0, :] = r["yout"]
        new_k[sl] = r["newk"]
        new_v[sl] = r["newv"]
    return out, new_k, new_v


# revision 18
# speedup vs baseline: 2.0002x; 1.3107x over previous
"""Trainium2 Bass kernel for nn_DecodeAttention (B=32, S=4095, E=1024, H=16).

Contract: kernel(**full_inputs) -> (out, new_k, new_v) as full-shape numpy
arrays. Internally shards batch across 8 NeuronCores (4 batches/core), runs
one SPMD Bass/Tile program via run_bass_kernel_spmd, gathers results.

Self-contained: hardcodes shapes; only imports the concourse toolchain from
its installed location.
"""

import os
import sys

if "/opt/trn_rl_repo" not in sys.path:
    sys.path.insert(0, "/opt/trn_rl_repo")

import numpy as np

B, S, E, H = 32, 4095, 1024, 16
HD = E // H            # 64
SEQ = S + 1            # 4096
NCORES = 8
BPC = B // NCORES      # 4 batches per core
CHUNK = 128            # rows per score sub-chunk (SBUF partitions)
NCHUNK = SEQ // CHUNK  # 32 sub-chunks per batch
CPB = 4                # sub-chunks per DMA block
SBLK = CHUNK * CPB     # 512 rows per DMA block
NBLK = NCHUNK // CPB   # 8 blocks per batch
NGRP = E // CHUNK      # 8 contraction groups of 128

_compiled = {}
LAST_EXEC_NS = None
LAST_RESULTS = None


def _build(lasts, mask_all_ones):
    """Build + compile the SPMD program for one core's 4 batches.

    lasts: tuple of BPC ints — scatter position per batch (baked into APs).
    mask_all_ones: if False, a maskcols input is declared and exp(scores) is
    multiplied by the per-position mask (equivalent to the -1e9 fill).
    """
    import concourse.tile as tile
    from concourse import bacc, mybir
    from concourse.masks import make_identity

    f32 = mybir.dt.float32
    AF = mybir.ActivationFunctionType
    AX = mybir.AxisListType

    nc = bacc.Bacc(
        "TRN2",
        target_bir_lowering=False,
        debug=False,
        enable_asserts=False,
        num_devices=NCORES,
    )

    kc = nc.dram_tensor("kcache", [BPC, SEQ, E], f32, kind="ExternalInput")
    vc = nc.dram_tensor("vcache", [BPC, SEQ, E], f32, kind="ExternalInput")
    hst = nc.dram_tensor("hst", [E, BPC], f32, kind="ExternalInput")
    wattn = nc.dram_tensor("wattn", [E, 3 * E], f32, kind="ExternalInput")
    battn = nc.dram_tensor("battn", [BPC, 3 * E], f32, kind="ExternalInput")
    wproj = nc.dram_tensor("wproj", [E, E], f32, kind="ExternalInput")
    bproj = nc.dram_tensor("bproj", [BPC, E], f32, kind="ExternalInput")
    biasc = nc.dram_tensor("biascols", [CHUNK, NCHUNK], f32, kind="ExternalInput")
    maskc = None
    if not mask_all_ones:
        maskc = nc.dram_tensor(
            "maskcols", [BPC, CHUNK, NCHUNK], f32, kind="ExternalInput"
        )

    newk = nc.dram_tensor("newk", [BPC, SEQ, E], f32, kind="ExternalOutput")
    newv = nc.dram_tensor("newv", [BPC, SEQ, E], f32, kind="ExternalOutput")
    yout = nc.dram_tensor("yout", [BPC, E], f32, kind="ExternalOutput")

    with tile.TileContext(nc) as tc:
        with (
            tc.tile_pool(name="consts", bufs=1) as consts,
            tc.tile_pool(name="wpool", bufs=1) as wpool,
            tc.tile_pool(name="kv", bufs=2) as kvpool,
            tc.tile_pool(name="tmp", bufs=2) as tmppool,
            tc.tile_pool(name="small", bufs=4) as small,
            tc.tile_pool(name="qp", bufs=2) as qpool,
        ):
            # ---- constants ----
            ones_col = consts.tile([CHUNK, 1], f32)
            nc.gpsimd.memset(ones_col[:], 1.0)
            ident = consts.tile([16, 16], f32)
            make_identity(nc, ident[:])
            biast = consts.tile([CHUNK, NCHUNK], f32)
            nc.sync.dma_start(biast[:], biasc[:])
            battn_t = consts.tile([BPC, 3 * E], f32)
            nc.sync.dma_start(battn_t[:], battn[:])
            bproj_t = consts.tile([BPC, E], f32)
            nc.sync.dma_start(bproj_t[:], bproj[:])
            hst_t = consts.tile([CHUNK, NGRP, BPC], f32)
            nc.sync.dma_start(
                hst_t[:], hst[:, :].rearrange("(g p) b -> p g b", p=CHUNK)
            )
            wproj_t = consts.tile([CHUNK, NGRP, E], f32)
            nc.sync.dma_start(
                wproj_t[:], wproj[:, :].rearrange("(g p) f -> p g f", p=CHUNK)
            )
            ctxT = consts.tile([CHUNK, NGRP * BPC], f32)
            qkv_sb = consts.tile([BPC, 3 * E], f32)

            # ---- fused qkv projection: qkv = hs @ Wattn + b ----
            # q columns first (4MB) so batch-0 attention can start early,
            # then the k/v columns (8MB).
            with tc.tile_pool(name="psqkv", bufs=1, space="PSUM") as psqkv:
                q_ps = psqkv.tile([BPC, E], f32, tag="q")
                for g in range(NGRP):
                    wq = wpool.tile([CHUNK, E], f32, tag="wq")
                    nc.sync.dma_start(
                        wq[:], wattn[g * CHUNK : (g + 1) * CHUNK, 0:E]
                    )
                    for n in range(2):
                        nc.tensor.matmul(
                            q_ps[:, n * 512 : (n + 1) * 512],
                            hst_t[:, g, :],
                            wq[:, n * 512 : (n + 1) * 512],
                            start=(g == 0),
                            stop=(g == NGRP - 1),
                        )
                nc.vector.tensor_add(
                    qkv_sb[:, 0:E], q_ps[:], battn_t[:, 0:E]
                )
                # replicate q_b across 128 partitions, all batches up front
                q_reps = []
                for b in range(BPC):
                    qstage = qpool.tile([1, E], f32, tag="qstage", bufs=1)
                    nc.sync.dma_start(qstage[:], qkv_sb[b : b + 1, 0:E])
                    q_rep = qpool.tile([CHUNK, E], f32, tag="qrep", bufs=BPC)
                    nc.gpsimd.partition_broadcast(q_rep[:], qstage[:])
                    q_reps.append(q_rep)

                kv_ps = psqkv.tile([BPC, 2 * E], f32, tag="kv")
                for g in range(NGRP):
                    wkv = wpool.tile([CHUNK, 2 * E], f32, tag="wkv")
                    nc.sync.dma_start(
                        wkv[:], wattn[g * CHUNK : (g + 1) * CHUNK, E : 3 * E]
                    )
                    for n in range(4):
                        nc.tensor.matmul(
                            kv_ps[:, n * 512 : (n + 1) * 512],
                            hst_t[:, g, :],
                            wkv[:, n * 512 : (n + 1) * 512],
                            start=(g == 0),
                            stop=(g == NGRP - 1),
                        )
                nc.vector.tensor_add(
                    qkv_sb[:, E : 3 * E], kv_ps[:], battn_t[:, E : 3 * E]
                )

            # ---- per-batch attention + cache update ----
            psctx_cm = tc.tile_pool(name="psctx", bufs=1, space="PSUM")
            psden_cm = tc.tile_pool(name="psden", bufs=1, space="PSUM")
            pstr_cm = tc.tile_pool(name="pstr", bufs=2, space="PSUM")
            psctx = psctx_cm.__enter__()
            psden = psden_cm.__enter__()
            pstr = pstr_cm.__enter__()
            for b in range(BPC):
                last = int(lasts[b])
                lblk, lrem = last // SBLK, last % SBLK
                # layout within a block: row r0 + p*CPB + r -> kt[p, r, :]
                lpart, lchunk = lrem // CPB, lrem % CPB

                q_rep = q_reps[b]

                if maskc is not None:
                    mk = qpool.tile([CHUNK, NCHUNK], f32, tag="mask")
                    nc.sync.dma_start(mk[:], maskc[b])

                ctx_ps = psctx.tile([16, E], f32)
                den_ps = psden.tile([16, 1], f32)

                for blk in range(NBLK):
                    r0 = blk * SBLK
                    kt = kvpool.tile([CHUNK, CPB, E], f32, tag="kt")
                    vt = kvpool.tile([CHUNK, CPB, E], f32, tag="vt")
                    # caches are host-padded to SEQ rows, so every block is a
                    # clean 128-partition DMA (16KB/partition descriptors).
                    nc.sync.dma_start(
                        kt[:],
                        kc[b, r0 : r0 + SBLK, :].rearrange(
                            "(p r) f -> p r f", p=CHUNK
                        ),
                    )
                    nc.sync.dma_start(
                        vt[:],
                        vc[b, r0 : r0 + SBLK, :].rearrange(
                            "(p r) f -> p r f", p=CHUNK
                        ),
                    )
                    if blk == NBLK - 1 and last != SEQ - 1:
                        # appended row must be zero unless scattered into
                        nc.gpsimd.memset(kt[CHUNK - 1 : CHUNK, CPB - 1, :], 0.0)
                        nc.gpsimd.memset(vt[CHUNK - 1 : CHUNK, CPB - 1, :], 0.0)
                    if blk == lblk:
                        # scatter new k/v row at position `last`
                        nc.sync.dma_start(
                            kt[lpart : lpart + 1, lchunk, :],
                            qkv_sb[b : b + 1, E : 2 * E],
                        )
                        nc.sync.dma_start(
                            vt[lpart : lpart + 1, lchunk, :],
                            qkv_sb[b : b + 1, 2 * E : 3 * E],
                        )

                    for c in range(CPB):
                        cg = blk * CPB + c
                        tmp = tmppool.tile([CHUNK, E], f32)
                        nc.vector.tensor_mul(tmp[:], kt[:, c, :], q_rep[:])
                        sc = small.tile([CHUNK, H], f32, tag="sc")
                        nc.vector.reduce_sum(
                            sc[:],
                            tmp[:].rearrange("p (h d) -> p h d", h=H),
                            axis=AX.X,
                        )
                        ex = small.tile([CHUNK, H], f32, tag="ex")
                        nc.scalar.activation(
                            ex[:],
                            sc[:],
                            AF.Exp,
                            bias=biast[:, cg : cg + 1],
                            scale=1.0 / 8.0,
                        )
                        if maskc is not None:
                            nc.vector.tensor_scalar_mul(
                                ex[:], ex[:], mk[:, cg : cg + 1]
                            )
                        st = cg == 0
                        sp = cg == NCHUNK - 1
                        nc.tensor.matmul(
                            ctx_ps[:, 0:512], ex[:], vt[:, c, 0:512],
                            start=st, stop=sp,
                        )
                        nc.tensor.matmul(
                            ctx_ps[:, 512:E], ex[:], vt[:, c, 512:E],
                            start=st, stop=sp,
                        )
                        nc.tensor.matmul(
                            den_ps[:], ex[:], ones_col[:], start=st, stop=sp
                        )

                    nc.scalar.dma_start(
                        newk[b, r0 : r0 + SBLK, :].rearrange(
                            "(p r) f -> p r f", p=CHUNK
                        ),
                        kt[:],
                    )
                    nc.scalar.dma_start(
                        newv[b, r0 : r0 + SBLK, :].rearrange(
                            "(p r) f -> p r f", p=CHUNK
                        ),
                        vt[:],
                    )

                # normalize: ctx[h, :] * (1 / denom[h]) via per-partition scale
                recip = small.tile([16, 1], f32, tag="recip", bufs=2)
                nc.vector.reciprocal(recip[:], den_ps[:])
                ctxn = small.tile([16, E], f32, tag="ctxn", bufs=2)
                nc.scalar.activation(
                    ctxn[:], ctx_ps[:], AF.Copy, bias=0.0, scale=recip[:]
                )
                # transpose [16, E] -> ctxT columns, keeping only each head's
                # own 64-dim block (diagonal extraction)
                for g in range(NGRP):
                    trp = pstr.tile([CHUNK, 16], f32)
                    nc.tensor.transpose(
                        trp[:], ctxn[:, g * CHUNK : (g + 1) * CHUNK], ident[:]
                    )
                    col = g * BPC + b
                    nc.vector.tensor_copy(
                        ctxT[0:HD, col : col + 1], trp[0:HD, 2 * g : 2 * g + 1]
                    )
                    nc.vector.tensor_copy(
                        ctxT[HD:CHUNK, col : col + 1],
                        trp[HD:CHUNK, 2 * g + 1 : 2 * g + 2],
                    )

            # ---- output projection for all 4 batches ----
            with tc.tile_pool(name="psy", bufs=1, space="PSUM") as psy:
                y_ps = psy.tile([BPC, E], f32, tag="y")
                for g in range(NGRP):
                    nc.tensor.matmul(
                        y_ps[:, 0:512],
                        ctxT[:, g * BPC : (g + 1) * BPC],
                        wproj_t[:, g, 0:512],
                        start=(g == 0),
                        stop=(g == NGRP - 1),
                    )
                    nc.tensor.matmul(
                        y_ps[:, 512:E],
                        ctxT[:, g * BPC : (g + 1) * BPC],
                        wproj_t[:, g, 512:E],
                        start=(g == 0),
                        stop=(g == NGRP - 1),
                    )
                y_sb = consts.tile([BPC, E], f32, tag="ysb")
                nc.vector.tensor_add(y_sb[:], y_ps[:], bproj_t[:])
                nc.sync.dma_start(yout[:], y_sb[:])
            pstr_cm.__exit__(None, None, None)
            psden_cm.__exit__(None, None, None)
            psctx_cm.__exit__(None, None, None)

    nc.compile()
    return nc


def get_program(lasts, mask_all_ones):
    key = (tuple(int(x) for x in lasts), bool(mask_all_ones))
    if key not in _compiled:
        _compiled[key] = _build(key[0], key[1])
    return _compiled[key]


def kernel(
    hidden_states,
    key_cache,
    value_cache,
    attention_mask,
    c_attn_w,
    c_attn_b,
    c_proj_w,
    c_proj_b,
    attn_bias,
):
    global LAST_EXEC_NS, LAST_RESULTS

    hs = np.ascontiguousarray(np.asarray(hidden_states, dtype=np.float32))
    # pad caches to SEQ rows so every device DMA is a uniform full-partition
    # transfer (the pad row is overwritten in SBUF by the scattered k/v row)
    kcache = np.zeros((B, SEQ, E), np.float32)
    kcache[:, :S] = np.asarray(key_cache, dtype=np.float32)
    vcache = np.zeros((B, SEQ, E), np.float32)
    vcache[:, :S] = np.asarray(value_cache, dtype=np.float32)
    mask = np.asarray(attention_mask)
    w_attn = np.ascontiguousarray(np.asarray(c_attn_w, dtype=np.float32))
    b_attn = np.asarray(c_attn_b, dtype=np.float32)
    w_proj = np.ascontiguousarray(np.asarray(c_proj_w, dtype=np.float32))
    b_proj = np.asarray(c_proj_b, dtype=np.float32)
    bias = np.asarray(attn_bias, dtype=np.float32)

    # last valid position per batch (index of last 1 in the mask)
    rev = mask[:, ::-1]
    last = mask.shape[1] - np.argmax(rev, axis=1) - 1
    mask_all_ones = bool((mask == 1).all())
    lasts_by_core = last.reshape(NCORES, BPC)
    if not (lasts_by_core == lasts_by_core[0]).all():
        raise NotImplementedError(
            "scatter positions must match across batch shards (SPMD program)"
        )

    nc = get_program(lasts_by_core[0], mask_all_ones)

    hstT = np.ascontiguousarray(hs[:, 0, :].T)  # [E, B]
    # biast[p, blk*CPB + r] = attn_bias[0, blk*SBLK + p*CPB + r]
    biascols = np.ascontiguousarray(
        bias[0, :SEQ].reshape(NBLK, CHUNK, CPB).transpose(1, 0, 2).reshape(CHUNK, NCHUNK)
    )
    battn = np.ascontiguousarray(np.tile(b_attn[None, :], (BPC, 1)))
    bprojt = np.ascontiguousarray(np.tile(b_proj[None, :], (BPC, 1)))
    if not mask_all_ones:
        maskcols = np.ascontiguousarray(
            mask[:, :SEQ]
            .reshape(B, NBLK, CHUNK, CPB)
            .transpose(0, 2, 1, 3)
            .reshape(B, CHUNK, NCHUNK)
            .astype(np.float32)
        )

    in_maps = []
    for i in range(NCORES):
        sl = slice(i * BPC, (i + 1) * BPC)
        m = {
            "kcache": kcache[sl],
            "vcache": vcache[sl],
            "hst": np.ascontiguousarray(hstT[:, sl]),
            "wattn": w_attn,
            "battn": battn,
            "wproj": w_proj,
            "bproj": bprojt,
            "biascols": biascols,
        }
        if not mask_all_ones:
            m["maskcols"] = maskcols[sl]
        in_maps.append(m)

    from concourse.bass_utils import run_bass_kernel_spmd

    trace = bool(int(os.environ.get("KERNEL_TRACE", "0")))
    res = run_bass_kernel_spmd(
        nc, in_maps, list(range(NCORES)), trace=trace
    )
    LAST_EXEC_NS = res.exec_time_ns
    LAST_RESULTS = res

    out = np.empty((B, 1, E), np.float32)
    new_k = np.empty((B, SEQ, E), np.float32)
    new_v = np.empty((B, SEQ, E), np.float32)
    for i, r in enumerate(res.results):
        sl = slice(i * BPC, (i + 1) * BPC)
        out[sl, 0, :] = r["yout"]
        new_k[sl] = r["newk"]
        new_v[sl] = r["newv"]
    return out, new_k, new_v
